# revision 21
# baseline (speedup 1.0000x reference)
# Trainium2 Bass kernel for a 2-layer bidirectional ESN (leaky-tanh RNN) encoder.
#
# Problem shape (hardcoded): x [64, 512, 80], lengths [64] (sorted desc,
# lens[0]=512), per-(layer,dir) W_hh [512,512], w_ih0 [512,80], w_ih1
# [512,1024].  Output: [64, 2048] = per-lane concat of last hidden states
# (layer0 fwd, layer0 bwd, layer1 fwd, layer1 bwd).
#
# Sharding: data-parallel over batch, 8 lanes per core, weights replicated.
# One SPMD program for all 8 cores; all length-dependence enters as *data*
# (masked inputs + mask tensors); last-state extraction happens host-side
# from dumped hidden-state history.
#
# Device algorithm per core (lanes b=0..7, chunks jc/kc=0..3 of H=512):
#   state p = h/LEAK stored fp16 in "hs" history buffers, layout
#     hs[p_row, t*32 + jc*8 + b]  (partition row = j within chunk)
#   per step:   psum[:, jc*8+b] += sum_kc (LEAK*W).T-tile @ hs[t-1] slice
#               u = psum + xp[t];  s = tanh(u);  p_new = 0.1*p + s
#               fwd only: p_new *= mask[t]        (zeros after seq end)
#   xp (input projections) precomputed on-device with big fp16 matmuls.
#   Backward direction needs no mask: padded xp is 0 so the state stays
#   exactly 0 until the sequence becomes active (matches packed semantics).

import os
import numpy as np

LEAK = 0.9
H = 512
I_IN = 80
L = 2
B = 64
T = 512
NCORES = 8
BL = B // NCORES          # 8 lanes per core
JC = H // 128             # 4 chunks
W32 = 4 * BL              # 32 = step block width (jc-major, lanes minor)

_CACHE = {}


def _build_program(t_steps=T):
    import concourse.bass as bass  # noqa: F401
    import concourse.tile as tile
    from concourse import bacc, mybir
    from contextlib import ExitStack

    dt = mybir.dt
    f16 = dt.float16
    f32 = dt.float32

    nc = bacc.Bacc("TRN2", target_bir_lowering=False, debug=False)

    TS = t_steps
    # ---- DRAM I/O (per-core shapes) ----
    xt_d = nc.dram_tensor("xt", [I_IN, TS * BL], f16, kind="ExternalInput")
    msk_d = nc.dram_tensor("msk", [128, TS * BL], f16, kind="ExternalInput")
    whh_d = nc.dram_tensor("whh", [128, L * 2 * JC * JC * 128], f16, kind="ExternalInput")
    wih0_d = nc.dram_tensor("wih0", [I_IN, 2 * H], f16, kind="ExternalInput")
    wih1_d = nc.dram_tensor("wih1", [128, 2 * (2 * JC) * H], f16, kind="ExternalInput")
    ident_d = nc.dram_tensor("ident", [128, 128], f16, kind="ExternalInput")
    hsf0_d = nc.dram_tensor("hsf0", [128, TS * W32], f16, kind="ExternalOutput")
    hsf1_d = nc.dram_tensor("hsf1", [128, TS * W32], f16, kind="ExternalOutput")
    lastb_d = nc.dram_tensor("lastb", [L, 128, W32], f16, kind="ExternalOutput")

    def whh_tile(l, d, kc, jc):
        col = (((l * 2 + d) * JC + kc) * JC + jc) * 128
        return whh_sb[:, col:col + 128]

    with tile.TileContext(nc) as tc, ExitStack() as ctx:
        const = ctx.enter_context(tc.tile_pool(name="const", bufs=1))
        xp_pool = ctx.enter_context(tc.tile_pool(name="xp", bufs=1))
        hs_pool = ctx.enter_context(tc.tile_pool(name="hs", bufs=1))
        work = ctx.enter_context(tc.tile_pool(name="work", bufs=4))
        psum = ctx.enter_context(tc.tile_pool(name="psum", bufs=3, space="PSUM"))
        psum_p = ctx.enter_context(tc.tile_pool(name="psum_p", bufs=2, space="PSUM"))

        xt_sb = const.tile([I_IN, TS * BL], f16, tag="xt")
        msk_sb = const.tile([128, TS * BL], f16, tag="msk")
        whh_sb = const.tile([128, L * 2 * JC * JC * 128], f16, tag="whh")
        wih0_sb = const.tile([I_IN, 2 * H], f16, tag="wih0")
        wih1_sb = const.tile([128, 2 * (2 * JC) * H], f16, tag="wih1")
        zeros = const.tile([128, W32], f16, tag="zeros")
        ident = const.tile([128, 128], f16, tag="ident")

        NTB0 = max(1, (TS * BL) // 512)
        TBW0 = min(512, TS * BL)
        for tb in range(NTB0):
            nc.sync.dma_start(xt_sb[:, tb * TBW0:(tb + 1) * TBW0],
                              xt_d[:, tb * TBW0:(tb + 1) * TBW0])
        nc.sync.dma_start(msk_sb[:], msk_d[:])
        nc.sync.dma_start(whh_sb[:], whh_d[:])
        nc.sync.dma_start(wih0_sb[:], wih0_d[:])
        nc.sync.dma_start(wih1_sb[:], wih1_d[:])
        nc.sync.dma_start(ident[:], ident_d[:])
        nc.vector.memset(zeros[:], 0.0)

        NTB = (TS * BL) // 512 if TS * BL >= 512 else 1
        TBW = min(512, TS * BL)          # projection column-block width

        def proj_block(l, d, tb, xp_t, hs_prev):
            """One projection block: xp for direction d, t-range tb (TBW cols)."""
            for jc in range(JC):
                ps = psum_p.tile([128, TBW], f32, tag="proj", name="proj_ps")
                if l == 0:
                    nc.tensor.matmul(
                        ps[:],
                        wih0_sb[:, d * H + jc * 128: d * H + jc * 128 + 128],
                        xt_sb[:, tb * TBW: (tb + 1) * TBW],
                        start=True, stop=True,
                    )
                else:
                    hsf_p, hsb_p = hs_prev
                    ntg = TBW // BL  # t-groups per block
                    for kc in range(2 * JC):
                        src = hsf_p if kc < JC else hsb_p
                        r = src[:].rearrange("p (t w) -> p t w", w=W32)
                        rhs = r[:, tb * ntg:(tb + 1) * ntg,
                                (kc % JC) * BL:(kc % JC) * BL + BL]
                        wcol = (d * 2 * JC + kc) * H + jc * 128
                        nc.tensor.matmul(
                            ps[:],
                            wih1_sb[:, wcol:wcol + 128],
                            rhs,
                            start=(kc == 0), stop=(kc == 2 * JC - 1),
                        )
                nc.vector.tensor_copy(
                    xp_t[:, jc * TS * BL + tb * TBW: jc * TS * BL + (tb + 1) * TBW],
                    ps[:],
                )

        def xp_ap(xp_t, t):
            r = xp_t[:].rearrange("p (j x) -> p j x", j=JC)
            return r[:, :, t * BL:(t + 1) * BL]

        def step_mm(l, d, t, prev, xp_t):
            """Matmul half of a step: returns accumulated psum tile."""
            ps = psum.tile([128, W32], f32, tag=f"ps{d}", name=f"ps{d}")
            # xp[t] enters PSUM via an identity matmul (frees DVE, shortens
            # the dependency chain: tanh reads PSUM directly)
            nc.tensor.matmul(
                ps[:].rearrange("p (j b) -> p j b", j=JC),
                ident[:],
                xp_ap(xp_t, t),
                start=True, stop=False, skip_group_check=True,
            )
            for jc in range(JC):
                for kc in range(JC):
                    nc.tensor.matmul(
                        ps[:, jc * BL:(jc + 1) * BL],
                        whh_tile(l, d, kc, jc),
                        prev[:, kc * BL:(kc + 1) * BL],
                        start=False, stop=(jc == JC - 1 and kc == JC - 1),
                        skip_group_check=True,
                    )
            return ps

        def step_chain(l, d, t, ps, prev, out_ap, masked):
            s = work.tile([128, W32], f16, tag=f"s{d}", name=f"s{d}")
            nc.scalar.activation(s[:], ps[:], mybir.ActivationFunctionType.Tanh)
            if masked:
                tmp = work.tile([128, W32], f16, tag=f"tmp{d}")
                nc.vector.scalar_tensor_tensor(
                    tmp[:], prev, 1.0 - LEAK, s[:],
                    op0=mybir.AluOpType.mult, op1=mybir.AluOpType.add,
                )
                m = msk_sb[:, t * BL:(t + 1) * BL].unsqueeze(1).broadcast_to((128, JC, BL))
                nc.vector.tensor_tensor(
                    out_ap.rearrange("p (j b) -> p j b", j=JC),
                    tmp[:].rearrange("p (j b) -> p j b", j=JC),
                    m,
                    mybir.AluOpType.mult,
                )
            else:
                nc.vector.scalar_tensor_tensor(
                    out_ap, prev, 1.0 - LEAK, s[:],
                    op0=mybir.AluOpType.mult, op1=mybir.AluOpType.add,
                )

        hs_prev = None
        TPB = TBW // BL  # recurrence steps covered per projection block
        for l in range(L):
            xp_f = xp_pool.tile([128, JC * TS * BL], f16, tag="xp0", name="xp0")
            xp_b = xp_pool.tile([128, JC * TS * BL], f16, tag="xp1", name="xp1")
            # Prologue: only the first block each direction consumes; the rest
            # are emitted inside the step loop so their matmuls fill PE gaps
            # in the recurrence instead of forming a serial phase.
            proj_block(l, 0, 0, xp_f, hs_prev)
            proj_block(l, 1, NTB - 1, xp_b, hs_prev)
            hsf = hs_pool.tile([128, TS * W32], f16, tag="hsf")
            hsb = hs_pool.tile([128, TS * W32], f16, tag="hsb")
            hsf_d = hsf0_d if l == 0 else hsf1_d
            nq = 4 if TS % 4 == 0 else 1
            for i in range(TS):
                if i % TPB == 0:
                    k = i // TPB
                    if k + 1 < NTB:
                        proj_block(l, 0, k + 1, xp_f, hs_prev)
                        proj_block(l, 1, NTB - 2 - k, xp_b, hs_prev)
                tf = i
                prev_f = zeros[:] if tf == 0 else hsf[:, (tf - 1) * W32: tf * W32]
                tb = TS - 1 - i
                prev_b = zeros[:] if i == 0 else hsb[:, (tb + 1) * W32:(tb + 2) * W32]
                # both dirs' matmuls adjacent, then both act/blend chains
                psf = step_mm(l, 0, tf, prev_f, xp_f)
                psb = step_mm(l, 1, tb, prev_b, xp_b)
                # Layer-1 fwd needs no masking: its post-end states feed
                # nothing (last state is gathered at len-1; no next layer).
                step_chain(l, 0, tf, psf, prev_f,
                           hsf[:, tf * W32:(tf + 1) * W32], l == 0)
                step_chain(l, 1, tb, psb, prev_b,
                           hsb[:, tb * W32:(tb + 1) * W32], False)
                if (i + 1) % (TS // nq) == 0:
                    q = (i + 1) // (TS // nq) - 1
                    c0, c1 = q * TS // nq * W32, (q + 1) * TS // nq * W32
                    nc.sync.dma_start(hsf_d[:, c0:c1], hsf[:, c0:c1])
            nc.sync.dma_start(lastb_d[l], hsb[:, 0:W32])
            hs_prev = (hsf, hsb)

    nc.compile()
    return nc


def _get_program(t_steps=T):
    if t_steps not in _CACHE:
        _CACHE[t_steps] = _build_program(t_steps)
    return _CACHE[t_steps]


def _prep_inputs(x, lengths, w_ih0_f, w_ih0_b, w_ih1_f, w_ih1_b, w_hh, t_steps=T):
    """Build per-core input maps (all host-side numpy)."""
    x = np.asarray(x, np.float32)
    lengths = np.asarray(lengths, np.int32)
    TS = t_steps

    # replicated weights
    whh = np.empty((128, L * 2 * JC * JC * 128), np.float16)
    w_hh = np.asarray(w_hh, np.float32)
    for l in range(L):
        for d in range(2):
            wt = (LEAK * w_hh[l, d]).T.astype(np.float16)  # [k, j]
            for kc in range(JC):
                for jc in range(JC):
                    col = (((l * 2 + d) * JC + kc) * JC + jc) * 128
                    whh[:, col:col + 128] = wt[kc * 128:(kc + 1) * 128,
                                               jc * 128:(jc + 1) * 128]
    wih0 = np.empty((I_IN, 2 * H), np.float16)
    wih0[:, 0:H] = np.asarray(w_ih0_f, np.float32).T.astype(np.float16)
    wih0[:, H:] = np.asarray(w_ih0_b, np.float32).T.astype(np.float16)
    wih1 = np.empty((128, 2 * (2 * JC) * H), np.float16)
    for d, w in enumerate((w_ih1_f, w_ih1_b)):
        wt = (LEAK * np.asarray(w, np.float32)).T.astype(np.float16)  # [1024, 512]
        for kc in range(2 * JC):
            wih1[:, (d * 2 * JC + kc) * H:(d * 2 * JC + kc + 1) * H] = \
                wt[kc * 128:(kc + 1) * 128, :]

    in_maps = []
    for c in range(NCORES):
        lanes = slice(c * BL, (c + 1) * BL)
        lens_c = lengths[lanes]                          # [BL]
        mask_c = (np.arange(TS)[:, None] < lens_c[None, :])  # [TS, BL]
        xm = x[lanes, :TS] * mask_c.T[:, :, None]        # [BL, TS, I]
        xt = np.ascontiguousarray(
            xm.transpose(2, 1, 0).reshape(I_IN, TS * BL)).astype(np.float16)
        msk = np.broadcast_to(
            mask_c.reshape(1, TS * BL), (128, TS * BL)).astype(np.float16).copy()
        in_maps.append({
            "xt": xt, "msk": msk, "whh": whh, "wih0": wih0, "wih1": wih1,
            "ident": np.eye(128, dtype=np.float16),
        })
    return in_maps


def _install_ntff_shim():
    """The agent image's antenv lacks axon_hooks; recreate it so
    run_bass_kernel_spmd(trace=True) can NTFF-profile via libaxon."""
    import sys, types
    if "antenv.axon_hooks" in sys.modules:
        return
    mod = types.ModuleType("antenv.axon_hooks")
    mod._hook = None
    mod.set_axon_ntff_profile_hook = lambda h: setattr(mod, "_hook", h)
    mod.get_axon_ntff_profile_hook = lambda: mod._hook
    sys.modules["antenv.axon_hooks"] = mod
    try:
        from trn_agent_boot.trn_boot import _ntff_profile_via_ctypes
        mod._hook = _ntff_profile_via_ctypes("/opt/axon/libaxon_pjrt.so")
    except Exception as e:  # degrade: no trace, run still works
        print(f"ntff shim failed: {e}")


def kernel(x, lengths, w_ih0_f, w_ih0_b, w_ih1_f, w_ih1_b, w_hh, _t_steps=None):
    from concourse.bass_utils import run_bass_kernel_spmd

    t_steps = _t_steps or T
    x = np.asarray(x, np.float32)
    lengths = np.asarray(lengths, np.int32)
    in_maps = _prep_inputs(x, lengths, w_ih0_f, w_ih0_b, w_ih1_f, w_ih1_b, w_hh,
                           t_steps)
    nc = _get_program(t_steps)

    trace = os.environ.get("KERNEL_TRACE", "0") == "1"
    kw = {}
    if trace:
        _install_ntff_shim()
        tmpdir = os.environ.get("KERNEL_TRACE_DIR") or "/tmp/kernel_trace"
        os.makedirs(tmpdir, exist_ok=True)
        kw = dict(trace=True, tmpdir=tmpdir)
    res = run_bass_kernel_spmd(nc, in_maps, list(range(NCORES)), **kw)
    if trace and res.exec_time_ns is not None:
        print(f"HW exec time: {res.exec_time_ns} ns")

    out = np.zeros((B, 2 * L * H), np.float32)
    jcs = np.arange(JC)
    for c in range(NCORES):
        r = res.results[c]
        hsf = [np.asarray(r["hsf0"], np.float32), np.asarray(r["hsf1"], np.float32)]
        lastb = np.asarray(r["lastb"], np.float32)
        for b in range(BL):
            g = c * BL + b
            ln = max(1, int(lengths[g]))
            for l in range(L):
                cols = (ln - 1) * W32 + jcs * BL + b
                f_last = hsf[l][:, cols].T.reshape(H)       # [jc,128] -> flat
                b_last = lastb[l][:, jcs * BL + b].T.reshape(H)
                out[g, (2 * l) * H:(2 * l + 1) * H] = f_last
                out[g, (2 * l + 1) * H:(2 * l + 2) * H] = b_last
    out *= LEAK  # state stored as p = h/LEAK
    return out


# revision 22
# speedup vs baseline: 1.1803x; 1.1803x over previous
# Trainium2 Bass kernel for a 2-layer bidirectional ESN (leaky-tanh RNN) encoder.
#
# Problem shape (hardcoded): x [64, 512, 80], lengths [64] (sorted desc,
# lens[0]=512), per-(layer,dir) W_hh [512,512], w_ih0 [512,80], w_ih1
# [512,1024].  Output: [64, 2048] = per-lane concat of last hidden states
# (layer0 fwd, layer0 bwd, layer1 fwd, layer1 bwd).
#
# Sharding: data-parallel over batch, 8 lanes per core, weights replicated.
# One SPMD program for all 8 cores; all length-dependence enters as *data*
# (masked inputs + mask tensors); last-state extraction happens host-side
# from dumped hidden-state history.
#
# Device algorithm per core (lanes b=0..7, chunks jc/kc=0..3 of H=512):
#   state p = h/LEAK stored fp16 in "hs" history buffers, layout
#     hs[p_row, t*32 + jc*8 + b]  (partition row = j within chunk)
#   per step:   psum[:, jc*8+b] += sum_kc (LEAK*W).T-tile @ hs[t-1] slice
#               u = psum + xp[t];  s = tanh(u);  p_new = 0.1*p + s
#               fwd only: p_new *= mask[t]        (zeros after seq end)
#   xp (input projections) precomputed on-device with big fp16 matmuls.
#   Backward direction needs no mask: padded xp is 0 so the state stays
#   exactly 0 until the sequence becomes active (matches packed semantics).

import os
import numpy as np

LEAK = 0.9
H = 512
I_IN = 80
L = 2
B = 64
T = 512
NCORES = 8
BL = B // NCORES          # 8 lanes per core
JC = H // 128             # 4 chunks
W32 = 4 * BL              # 32 = step block width (jc-major, lanes minor)

_CACHE = {}


def _build_program(t_steps=T):
    import concourse.bass as bass  # noqa: F401
    import concourse.tile as tile
    from concourse import bacc, mybir
    from contextlib import ExitStack

    dt = mybir.dt
    f16 = dt.float16
    f32 = dt.float32

    nc = bacc.Bacc("TRN2", target_bir_lowering=False, debug=False)

    TS = t_steps
    # ---- DRAM I/O (per-core shapes) ----
    xt_d = nc.dram_tensor("xt", [I_IN, TS * BL], f16, kind="ExternalInput")
    msk_d = nc.dram_tensor("msk", [128, TS * BL], f16, kind="ExternalInput")
    whh_d = nc.dram_tensor("whh", [128, L * 2 * JC * JC * 128], f16, kind="ExternalInput")
    wih0_d = nc.dram_tensor("wih0", [I_IN, 2 * H], f16, kind="ExternalInput")
    wih1_d = nc.dram_tensor("wih1", [128, 2 * (2 * JC) * H], f16, kind="ExternalInput")
    ident_d = nc.dram_tensor("ident", [128, 128], f16, kind="ExternalInput")
    hsf0_d = nc.dram_tensor("hsf0", [128, TS * W32], f16, kind="ExternalOutput")
    hsf1_d = nc.dram_tensor("hsf1", [128, TS * W32], f16, kind="ExternalOutput")
    lastb_d = nc.dram_tensor("lastb", [L, 128, W32], f16, kind="ExternalOutput")

    def whh_tile(l, d, kc, jc):
        col = (((l * 2 + d) * JC + kc) * JC + jc) * 128
        return whh_sb[:, col:col + 128]

    with tile.TileContext(nc) as tc, ExitStack() as ctx:
        const = ctx.enter_context(tc.tile_pool(name="const", bufs=1))
        xp_pool = ctx.enter_context(tc.tile_pool(name="xp", bufs=1))
        hs_pool = ctx.enter_context(tc.tile_pool(name="hs", bufs=1))
        work = ctx.enter_context(tc.tile_pool(name="work", bufs=4))
        psum = ctx.enter_context(tc.tile_pool(name="psum", bufs=3, space="PSUM"))
        psum_p = ctx.enter_context(tc.tile_pool(name="psum_p", bufs=2, space="PSUM"))

        xt_sb = const.tile([I_IN, TS * BL], f16, tag="xt")
        msk_sb = const.tile([128, TS * BL], f16, tag="msk")
        whh_sb = const.tile([128, L * 2 * JC * JC * 128], f16, tag="whh")
        wih0_sb = const.tile([I_IN, 2 * H], f16, tag="wih0")
        wih1_sb = const.tile([128, 2 * (2 * JC) * H], f16, tag="wih1")
        zeros = const.tile([128, W32], f16, tag="zeros")
        ident = const.tile([128, 128], f16, tag="ident")

        NTB0 = max(1, (TS * BL) // 512)
        TBW0 = min(512, TS * BL)
        for tb in range(NTB0):
            nc.sync.dma_start(xt_sb[:, tb * TBW0:(tb + 1) * TBW0],
                              xt_d[:, tb * TBW0:(tb + 1) * TBW0])
        nc.sync.dma_start(msk_sb[:], msk_d[:])
        nc.sync.dma_start(whh_sb[:], whh_d[:])
        nc.sync.dma_start(wih0_sb[:], wih0_d[:])
        nc.sync.dma_start(wih1_sb[:], wih1_d[:])
        nc.sync.dma_start(ident[:], ident_d[:])
        nc.vector.memset(zeros[:], 0.0)

        NTB = (TS * BL) // 512 if TS * BL >= 512 else 1
        TBW = min(512, TS * BL)          # projection column-block width

        def projection(l, hs_prev):
            """Compute xp tiles [128, JC*TS*BL] fp16 for both dirs of layer l."""
            xps = []
            for d in range(2):
                xp_t = xp_pool.tile([128, JC * TS * BL], f16, tag=f"xp{d}")
                for jc in range(JC):
                    for tb in range(NTB):
                        ps = psum_p.tile([128, TBW], f32, tag="proj")
                        if l == 0:
                            nc.tensor.matmul(
                                ps[:],
                                wih0_sb[:, d * H + jc * 128: d * H + jc * 128 + 128],
                                xt_sb[:, tb * TBW: (tb + 1) * TBW],
                                start=True, stop=True,
                            )
                        else:
                            hsf_p, hsb_p = hs_prev
                            ntg = TBW // BL  # t-groups per block
                            for kc in range(2 * JC):
                                src = hsf_p if kc < JC else hsb_p
                                r = src[:].rearrange("p (t w) -> p t w", w=W32)
                                rhs = r[:, tb * ntg:(tb + 1) * ntg,
                                        (kc % JC) * BL:(kc % JC) * BL + BL]
                                wcol = (d * 2 * JC + kc) * H + jc * 128
                                nc.tensor.matmul(
                                    ps[:],
                                    wih1_sb[:, wcol:wcol + 128],
                                    rhs,
                                    start=(kc == 0), stop=(kc == 2 * JC - 1),
                                )
                        nc.vector.tensor_copy(
                            xp_t[:, jc * TS * BL + tb * TBW: jc * TS * BL + (tb + 1) * TBW],
                            ps[:],
                        )
                xps.append(xp_t)
            return xps

        def xp_ap(xp_t, t):
            r = xp_t[:].rearrange("p (j x) -> p j x", j=JC)
            return r[:, :, t * BL:(t + 1) * BL]

        def step_mm(l, d, t, prev, xp_t):
            """Matmul half of a step: returns accumulated psum tile."""
            ps = psum.tile([128, W32], f32, tag=f"ps{d}", name=f"ps{d}")
            # xp[t] enters PSUM via an identity matmul (frees DVE, shortens
            # the dependency chain: tanh reads PSUM directly)
            nc.tensor.matmul(
                ps[:].rearrange("p (j b) -> p j b", j=JC),
                ident[:],
                xp_ap(xp_t, t),
                start=True, stop=False, skip_group_check=True,
            )
            for jc in range(JC):
                for kc in range(JC):
                    nc.tensor.matmul(
                        ps[:, jc * BL:(jc + 1) * BL],
                        whh_tile(l, d, kc, jc),
                        prev[:, kc * BL:(kc + 1) * BL],
                        start=False, stop=(jc == JC - 1 and kc == JC - 1),
                        skip_group_check=True,
                    )
            return ps

        def step_chain(l, d, t, ps, prev, out_ap, masked):
            s = work.tile([128, W32], f16, tag=f"s{d}", name=f"s{d}")
            nc.scalar.activation(s[:], ps[:], mybir.ActivationFunctionType.Tanh)
            if masked:
                tmp = work.tile([128, W32], f16, tag=f"tmp{d}")
                nc.vector.scalar_tensor_tensor(
                    tmp[:], prev, 1.0 - LEAK, s[:],
                    op0=mybir.AluOpType.mult, op1=mybir.AluOpType.add,
                )
                m = msk_sb[:, t * BL:(t + 1) * BL].unsqueeze(1).broadcast_to((128, JC, BL))
                nc.vector.tensor_tensor(
                    out_ap.rearrange("p (j b) -> p j b", j=JC),
                    tmp[:].rearrange("p (j b) -> p j b", j=JC),
                    m,
                    mybir.AluOpType.mult,
                )
            else:
                nc.vector.scalar_tensor_tensor(
                    out_ap, prev, 1.0 - LEAK, s[:],
                    op0=mybir.AluOpType.mult, op1=mybir.AluOpType.add,
                )

        hs_prev = None
        for l in range(L):
            xp_f, xp_b = projection(l, hs_prev)
            hsf = hs_pool.tile([128, TS * W32], f16, tag="hsf")
            hsb = hs_pool.tile([128, TS * W32], f16, tag="hsb")
            hsf_d = hsf0_d if l == 0 else hsf1_d
            nq = 4 if TS % 4 == 0 else 1
            for i in range(TS):
                tf = i
                prev_f = zeros[:] if tf == 0 else hsf[:, (tf - 1) * W32: tf * W32]
                tb = TS - 1 - i
                prev_b = zeros[:] if i == 0 else hsb[:, (tb + 1) * W32:(tb + 2) * W32]
                # both dirs' matmuls adjacent, then both act/blend chains
                psf = step_mm(l, 0, tf, prev_f, xp_f)
                psb = step_mm(l, 1, tb, prev_b, xp_b)
                # Layer-1 fwd needs no masking: its post-end states feed
                # nothing (last state is gathered at len-1; no next layer).
                step_chain(l, 0, tf, psf, prev_f,
                           hsf[:, tf * W32:(tf + 1) * W32], l == 0)
                step_chain(l, 1, tb, psb, prev_b,
                           hsb[:, tb * W32:(tb + 1) * W32], False)
                if (i + 1) % (TS // nq) == 0:
                    q = (i + 1) // (TS // nq) - 1
                    c0, c1 = q * TS // nq * W32, (q + 1) * TS // nq * W32
                    nc.sync.dma_start(hsf_d[:, c0:c1], hsf[:, c0:c1])
            nc.sync.dma_start(lastb_d[l], hsb[:, 0:W32])
            hs_prev = (hsf, hsb)

    nc.compile()
    return nc


def _get_program(t_steps=T):
    if t_steps not in _CACHE:
        _CACHE[t_steps] = _build_program(t_steps)
    return _CACHE[t_steps]


def _prep_inputs(x, lengths, w_ih0_f, w_ih0_b, w_ih1_f, w_ih1_b, w_hh, t_steps=T):
    """Build per-core input maps (all host-side numpy)."""
    x = np.asarray(x, np.float32)
    lengths = np.asarray(lengths, np.int32)
    TS = t_steps

    # replicated weights
    whh = np.empty((128, L * 2 * JC * JC * 128), np.float16)
    w_hh = np.asarray(w_hh, np.float32)
    for l in range(L):
        for d in range(2):
            wt = (LEAK * w_hh[l, d]).T.astype(np.float16)  # [k, j]
            for kc in range(JC):
                for jc in range(JC):
                    col = (((l * 2 + d) * JC + kc) * JC + jc) * 128
                    whh[:, col:col + 128] = wt[kc * 128:(kc + 1) * 128,
                                               jc * 128:(jc + 1) * 128]
    wih0 = np.empty((I_IN, 2 * H), np.float16)
    wih0[:, 0:H] = np.asarray(w_ih0_f, np.float32).T.astype(np.float16)
    wih0[:, H:] = np.asarray(w_ih0_b, np.float32).T.astype(np.float16)
    wih1 = np.empty((128, 2 * (2 * JC) * H), np.float16)
    for d, w in enumerate((w_ih1_f, w_ih1_b)):
        wt = (LEAK * np.asarray(w, np.float32)).T.astype(np.float16)  # [1024, 512]
        for kc in range(2 * JC):
            wih1[:, (d * 2 * JC + kc) * H:(d * 2 * JC + kc + 1) * H] = \
                wt[kc * 128:(kc + 1) * 128, :]

    in_maps = []
    for c in range(NCORES):
        lanes = slice(c * BL, (c + 1) * BL)
        lens_c = lengths[lanes]                          # [BL]
        mask_c = (np.arange(TS)[:, None] < lens_c[None, :])  # [TS, BL]
        xm = x[lanes, :TS] * mask_c.T[:, :, None]        # [BL, TS, I]
        xt = np.ascontiguousarray(
            xm.transpose(2, 1, 0).reshape(I_IN, TS * BL)).astype(np.float16)
        msk = np.broadcast_to(
            mask_c.reshape(1, TS * BL), (128, TS * BL)).astype(np.float16).copy()
        in_maps.append({
            "xt": xt, "msk": msk, "whh": whh, "wih0": wih0, "wih1": wih1,
            "ident": np.eye(128, dtype=np.float16),
        })
    return in_maps


def _install_ntff_shim():
    """The agent image's antenv lacks axon_hooks; recreate it so
    run_bass_kernel_spmd(trace=True) can NTFF-profile via libaxon."""
    import sys, types
    if "antenv.axon_hooks" in sys.modules:
        return
    mod = types.ModuleType("antenv.axon_hooks")
    mod._hook = None
    mod.set_axon_ntff_profile_hook = lambda h: setattr(mod, "_hook", h)
    mod.get_axon_ntff_profile_hook = lambda: mod._hook
    sys.modules["antenv.axon_hooks"] = mod
    try:
        from trn_agent_boot.trn_boot import _ntff_profile_via_ctypes
        mod._hook = _ntff_profile_via_ctypes("/opt/axon/libaxon_pjrt.so")
    except Exception as e:  # degrade: no trace, run still works
        print(f"ntff shim failed: {e}")


def kernel(x, lengths, w_ih0_f, w_ih0_b, w_ih1_f, w_ih1_b, w_hh, _t_steps=None):
    from concourse.bass_utils import run_bass_kernel_spmd

    t_steps = _t_steps or T
    x = np.asarray(x, np.float32)
    lengths = np.asarray(lengths, np.int32)
    in_maps = _prep_inputs(x, lengths, w_ih0_f, w_ih0_b, w_ih1_f, w_ih1_b, w_hh,
                           t_steps)
    nc = _get_program(t_steps)

    trace = os.environ.get("KERNEL_TRACE", "0") == "1"
    kw = {}
    if trace:
        _install_ntff_shim()
        tmpdir = os.environ.get("KERNEL_TRACE_DIR") or "/tmp/kernel_trace"
        os.makedirs(tmpdir, exist_ok=True)
        kw = dict(trace=True, tmpdir=tmpdir)
    res = run_bass_kernel_spmd(nc, in_maps, list(range(NCORES)), **kw)
    if trace and res.exec_time_ns is not None:
        print(f"HW exec time: {res.exec_time_ns} ns")

    out = np.zeros((B, 2 * L * H), np.float32)
    jcs = np.arange(JC)
    for c in range(NCORES):
        r = res.results[c]
        hsf = [np.asarray(r["hsf0"], np.float32), np.asarray(r["hsf1"], np.float32)]
        lastb = np.asarray(r["lastb"], np.float32)
        for b in range(BL):
            g = c * BL + b
            ln = max(1, int(lengths[g]))
            for l in range(L):
                cols = (ln - 1) * W32 + jcs * BL + b
                f_last = hsf[l][:, cols].T.reshape(H)       # [jc,128] -> flat
                b_last = lastb[l][:, jcs * BL + b].T.reshape(H)
                out[g, (2 * l) * H:(2 * l + 1) * H] = f_last
                out[g, (2 * l + 1) * H:(2 * l + 2) * H] = b_last
    out *= LEAK  # state stored as p = h/LEAK
    return out


# revision 23
# speedup vs baseline: 1.3292x; 1.1261x over previous
# Trainium2 Bass kernel for a 2-layer bidirectional ESN (leaky-tanh RNN) encoder.
#
# Problem shape (hardcoded): x [64, 512, 80], lengths [64] (sorted desc,
# lens[0]=512), per-(layer,dir) W_hh [512,512], w_ih0 [512,80], w_ih1
# [512,1024].  Output: [64, 2048] = per-lane concat of last hidden states
# (layer0 fwd, layer0 bwd, layer1 fwd, layer1 bwd).
#
# Sharding: data-parallel over batch, 8 lanes per core, weights replicated.
# One SPMD program for all 8 cores; all length-dependence enters as *data*
# (masked inputs + mask tensors); last-state extraction happens host-side
# from dumped hidden-state history.
#
# Device algorithm per core (lanes b=0..7, chunks jc/kc=0..3 of H=512):
#   state p = h/LEAK stored fp16 in "hs" history buffers, layout
#     hs[p_row, t*32 + jc*8 + b]  (partition row = j within chunk)
#   per step:   psum[:, jc*8+b] += sum_kc (LEAK*W).T-tile @ hs[t-1] slice
#               u = psum + xp[t];  s = tanh(u);  p_new = 0.1*p + s
#               fwd only: p_new *= mask[t]        (zeros after seq end)
#   xp (input projections) precomputed on-device with big fp16 matmuls.
#   Backward direction needs no mask: padded xp is 0 so the state stays
#   exactly 0 until the sequence becomes active (matches packed semantics).

import os
import numpy as np

LEAK = 0.9
H = 512
I_IN = 80
L = 2
B = 64
T = 512
NCORES = 8
BL = B // NCORES          # 8 lanes per core
JC = H // 128             # 4 chunks
W32 = 4 * BL              # 32 = step block width (jc-major, lanes minor)

_CACHE = {}


def _build_program(t_steps=T):
    import concourse.bass as bass  # noqa: F401
    import concourse.tile as tile
    from concourse import bacc, mybir
    from contextlib import ExitStack

    dt = mybir.dt
    f16 = dt.float16
    f32 = dt.float32

    nc = bacc.Bacc("TRN2", target_bir_lowering=False, debug=False)

    TS = t_steps
    # ---- DRAM I/O (per-core shapes) ----
    xt_d = nc.dram_tensor("xt", [I_IN, TS * BL], f16, kind="ExternalInput")
    msk_d = nc.dram_tensor("msk", [128, TS * BL], f16, kind="ExternalInput")
    whh_d = nc.dram_tensor("whh", [128, L * 2 * JC * JC * 128], f16, kind="ExternalInput")
    wih0_d = nc.dram_tensor("wih0", [I_IN, 2 * H], f16, kind="ExternalInput")
    wih1_d = nc.dram_tensor("wih1", [128, 2 * (2 * JC) * H], f16, kind="ExternalInput")
    ident_d = nc.dram_tensor("ident", [128, 128], f16, kind="ExternalInput")
    hsf0_d = nc.dram_tensor("hsf0", [128, TS * W32], f16, kind="ExternalOutput")
    hsf1_d = nc.dram_tensor("hsf1", [128, TS * W32], f16, kind="ExternalOutput")
    lastb_d = nc.dram_tensor("lastb", [L, 128, W32], f16, kind="ExternalOutput")

    def whh_tile(l, d, kc, jc):
        col = (((l * 2 + d) * JC + kc) * JC + jc) * 128
        return whh_sb[:, col:col + 128]

    with tile.TileContext(nc) as tc, ExitStack() as ctx:
        const = ctx.enter_context(tc.tile_pool(name="const", bufs=1))
        xp_pool = ctx.enter_context(tc.tile_pool(name="xp", bufs=1))
        hs_pool = ctx.enter_context(tc.tile_pool(name="hs", bufs=1))
        work = ctx.enter_context(tc.tile_pool(name="work", bufs=4))
        psum = ctx.enter_context(tc.tile_pool(name="psum", bufs=3, space="PSUM"))
        psum_p = ctx.enter_context(tc.tile_pool(name="psum_p", bufs=2, space="PSUM"))

        xt_sb = const.tile([I_IN, TS * BL], f16, tag="xt")
        msk_sb = const.tile([128, TS * BL], f16, tag="msk")
        whh_sb = const.tile([128, L * 2 * JC * JC * 128], f16, tag="whh")
        wih0_sb = const.tile([I_IN, 2 * H], f16, tag="wih0")
        wih1_sb = const.tile([128, 2 * (2 * JC) * H], f16, tag="wih1")
        zeros = const.tile([128, W32], f16, tag="zeros")
        ident = const.tile([128, 128], f16, tag="ident")

        NTB0 = max(1, (TS * BL) // 512)
        TBW0 = min(512, TS * BL)
        for tb in range(NTB0):
            nc.sync.dma_start(xt_sb[:, tb * TBW0:(tb + 1) * TBW0],
                              xt_d[:, tb * TBW0:(tb + 1) * TBW0])
        nc.sync.dma_start(msk_sb[:], msk_d[:])
        nc.sync.dma_start(whh_sb[:], whh_d[:])
        nc.sync.dma_start(wih0_sb[:], wih0_d[:])
        nc.sync.dma_start(wih1_sb[:], wih1_d[:])
        nc.sync.dma_start(ident[:], ident_d[:])
        nc.vector.memset(zeros[:], 0.0)

        NTB = (TS * BL) // 512 if TS * BL >= 512 else 1
        TBW = min(512, TS * BL)          # projection column-block width

        def projection(l, hs_prev):
            """Compute xp tiles [128, JC*TS*BL] fp16 for both dirs of layer l."""
            xps = []
            for d in range(2):
                xp_t = xp_pool.tile([128, JC * TS * BL], f16, tag=f"xp{d}")
                for jc in range(JC):
                    for tb in range(NTB):
                        ps = psum_p.tile([128, TBW], f32, tag="proj")
                        if l == 0:
                            nc.tensor.matmul(
                                ps[:],
                                wih0_sb[:, d * H + jc * 128: d * H + jc * 128 + 128],
                                xt_sb[:, tb * TBW: (tb + 1) * TBW],
                                start=True, stop=True,
                            )
                        else:
                            hsf_p, hsb_p = hs_prev
                            ntg = TBW // BL  # t-groups per block
                            for kc in range(2 * JC):
                                src = hsf_p if kc < JC else hsb_p
                                r = src[:].rearrange("p (t w) -> p t w", w=W32)
                                rhs = r[:, tb * ntg:(tb + 1) * ntg,
                                        (kc % JC) * BL:(kc % JC) * BL + BL]
                                wcol = (d * 2 * JC + kc) * H + jc * 128
                                nc.tensor.matmul(
                                    ps[:],
                                    wih1_sb[:, wcol:wcol + 128],
                                    rhs,
                                    start=(kc == 0), stop=(kc == 2 * JC - 1),
                                )
                        nc.vector.tensor_copy(
                            xp_t[:, jc * TS * BL + tb * TBW: jc * TS * BL + (tb + 1) * TBW],
                            ps[:],
                        )
                xps.append(xp_t)
            return xps

        def xp_ap(xp_t, t):
            r = xp_t[:].rearrange("p (j x) -> p j x", j=JC)
            return r[:, :, t * BL:(t + 1) * BL]

        def step_mm(l, d, t, prev, xp_t):
            """Matmul half of a step: returns accumulated psum tile."""
            ps = psum.tile([128, W32], f32, tag=f"ps{d}", name=f"ps{d}")
            # xp[t] enters PSUM via an identity matmul (frees DVE, shortens
            # the dependency chain: tanh reads PSUM directly)
            nc.tensor.matmul(
                ps[:].rearrange("p (j b) -> p j b", j=JC),
                ident[:],
                xp_ap(xp_t, t),
                start=True, stop=False, skip_group_check=True,
            )
            for jc in range(JC):
                for kc in range(JC):
                    nc.tensor.matmul(
                        ps[:, jc * BL:(jc + 1) * BL],
                        whh_tile(l, d, kc, jc),
                        prev[:, kc * BL:(kc + 1) * BL],
                        start=False, stop=(jc == JC - 1 and kc == JC - 1),
                        skip_group_check=True,
                    )
            return ps

        def step_chain(l, d, t, ps, prev, out_ap, masked):
            """Returns the AP the next step should read as its state."""
            s = work.tile([128, W32], f16, tag=f"s{d}", name=f"s{d}")
            nc.scalar.activation(s[:], ps[:], mybir.ActivationFunctionType.Tanh)
            if masked:
                # Iterate on the UNMASKED state (pu): ended lanes drift with
                # bounded garbage that is never read (their history writes are
                # masked to 0 below, off the loop-carried critical chain).
                pu = work.tile([128, W32], f16, tag=f"pu{d}", name=f"pu{d}")
                nc.vector.scalar_tensor_tensor(
                    pu[:], prev, 1.0 - LEAK, s[:],
                    op0=mybir.AluOpType.mult, op1=mybir.AluOpType.add,
                )
                m = msk_sb[:, t * BL:(t + 1) * BL].unsqueeze(1).broadcast_to((128, JC, BL))
                nc.vector.tensor_tensor(
                    out_ap.rearrange("p (j b) -> p j b", j=JC),
                    pu[:].rearrange("p (j b) -> p j b", j=JC),
                    m,
                    mybir.AluOpType.mult,
                )
                return pu[:]
            else:
                nc.vector.scalar_tensor_tensor(
                    out_ap, prev, 1.0 - LEAK, s[:],
                    op0=mybir.AluOpType.mult, op1=mybir.AluOpType.add,
                )
                return out_ap

        hs_prev = None
        for l in range(L):
            xp_f, xp_b = projection(l, hs_prev)
            hsf = hs_pool.tile([128, TS * W32], f16, tag="hsf")
            hsb = hs_pool.tile([128, TS * W32], f16, tag="hsb")
            hsf_d = hsf0_d if l == 0 else hsf1_d
            nq = 4 if TS % 4 == 0 else 1
            prev_f = zeros[:]
            prev_b = zeros[:]
            for i in range(TS):
                tf = i
                tb = TS - 1 - i
                # both dirs' matmuls adjacent, then both act/blend chains
                psf = step_mm(l, 0, tf, prev_f, xp_f)
                psb = step_mm(l, 1, tb, prev_b, xp_b)
                # Layer-1 fwd needs no masking: its post-end states feed
                # nothing (last state is gathered at len-1; no next layer).
                prev_f = step_chain(l, 0, tf, psf, prev_f,
                                    hsf[:, tf * W32:(tf + 1) * W32], l == 0)
                prev_b = step_chain(l, 1, tb, psb, prev_b,
                                    hsb[:, tb * W32:(tb + 1) * W32], False)
                if (i + 1) % (TS // nq) == 0:
                    q = (i + 1) // (TS // nq) - 1
                    c0, c1 = q * TS // nq * W32, (q + 1) * TS // nq * W32
                    nc.sync.dma_start(hsf_d[:, c0:c1], hsf[:, c0:c1])
            nc.sync.dma_start(lastb_d[l], hsb[:, 0:W32])
            hs_prev = (hsf, hsb)

    nc.compile()
    return nc


def _get_program(t_steps=T):
    if t_steps not in _CACHE:
        _CACHE[t_steps] = _build_program(t_steps)
    return _CACHE[t_steps]


def _prep_inputs(x, lengths, w_ih0_f, w_ih0_b, w_ih1_f, w_ih1_b, w_hh, t_steps=T):
    """Build per-core input maps (all host-side numpy)."""
    x = np.asarray(x, np.float32)
    lengths = np.asarray(lengths, np.int32)
    TS = t_steps

    # replicated weights
    whh = np.empty((128, L * 2 * JC * JC * 128), np.float16)
    w_hh = np.asarray(w_hh, np.float32)
    for l in range(L):
        for d in range(2):
            wt = (LEAK * w_hh[l, d]).T.astype(np.float16)  # [k, j]
            for kc in range(JC):
                for jc in range(JC):
                    col = (((l * 2 + d) * JC + kc) * JC + jc) * 128
                    whh[:, col:col + 128] = wt[kc * 128:(kc + 1) * 128,
                                               jc * 128:(jc + 1) * 128]
    wih0 = np.empty((I_IN, 2 * H), np.float16)
    wih0[:, 0:H] = np.asarray(w_ih0_f, np.float32).T.astype(np.float16)
    wih0[:, H:] = np.asarray(w_ih0_b, np.float32).T.astype(np.float16)
    wih1 = np.empty((128, 2 * (2 * JC) * H), np.float16)
    for d, w in enumerate((w_ih1_f, w_ih1_b)):
        wt = (LEAK * np.asarray(w, np.float32)).T.astype(np.float16)  # [1024, 512]
        for kc in range(2 * JC):
            wih1[:, (d * 2 * JC + kc) * H:(d * 2 * JC + kc + 1) * H] = \
                wt[kc * 128:(kc + 1) * 128, :]

    in_maps = []
    for c in range(NCORES):
        lanes = slice(c * BL, (c + 1) * BL)
        lens_c = lengths[lanes]                          # [BL]
        mask_c = (np.arange(TS)[:, None] < lens_c[None, :])  # [TS, BL]
        xm = x[lanes, :TS] * mask_c.T[:, :, None]        # [BL, TS, I]
        xt = np.ascontiguousarray(
            xm.transpose(2, 1, 0).reshape(I_IN, TS * BL)).astype(np.float16)
        msk = np.broadcast_to(
            mask_c.reshape(1, TS * BL), (128, TS * BL)).astype(np.float16).copy()
        in_maps.append({
            "xt": xt, "msk": msk, "whh": whh, "wih0": wih0, "wih1": wih1,
            "ident": np.eye(128, dtype=np.float16),
        })
    return in_maps


def _install_ntff_shim():
    """The agent image's antenv lacks axon_hooks; recreate it so
    run_bass_kernel_spmd(trace=True) can NTFF-profile via libaxon."""
    import sys, types
    if "antenv.axon_hooks" in sys.modules:
        return
    mod = types.ModuleType("antenv.axon_hooks")
    mod._hook = None
    mod.set_axon_ntff_profile_hook = lambda h: setattr(mod, "_hook", h)
    mod.get_axon_ntff_profile_hook = lambda: mod._hook
    sys.modules["antenv.axon_hooks"] = mod
    try:
        from trn_agent_boot.trn_boot import _ntff_profile_via_ctypes
        mod._hook = _ntff_profile_via_ctypes("/opt/axon/libaxon_pjrt.so")
    except Exception as e:  # degrade: no trace, run still works
        print(f"ntff shim failed: {e}")


def kernel(x, lengths, w_ih0_f, w_ih0_b, w_ih1_f, w_ih1_b, w_hh, _t_steps=None):
    from concourse.bass_utils import run_bass_kernel_spmd

    t_steps = _t_steps or T
    x = np.asarray(x, np.float32)
    lengths = np.asarray(lengths, np.int32)
    in_maps = _prep_inputs(x, lengths, w_ih0_f, w_ih0_b, w_ih1_f, w_ih1_b, w_hh,
                           t_steps)
    nc = _get_program(t_steps)

    trace = os.environ.get("KERNEL_TRACE", "0") == "1"
    kw = {}
    if trace:
        _install_ntff_shim()
        tmpdir = os.environ.get("KERNEL_TRACE_DIR") or "/tmp/kernel_trace"
        os.makedirs(tmpdir, exist_ok=True)
        kw = dict(trace=True, tmpdir=tmpdir)
    res = run_bass_kernel_spmd(nc, in_maps, list(range(NCORES)), **kw)
    if trace and res.exec_time_ns is not None:
        print(f"HW exec time: {res.exec_time_ns} ns")

    out = np.zeros((B, 2 * L * H), np.float32)
    jcs = np.arange(JC)
    for c in range(NCORES):
        r = res.results[c]
        hsf = [np.asarray(r["hsf0"], np.float32), np.asarray(r["hsf1"], np.float32)]
        lastb = np.asarray(r["lastb"], np.float32)
        for b in range(BL):
            g = c * BL + b
            ln = max(1, int(lengths[g]))
            for l in range(L):
                cols = (ln - 1) * W32 + jcs * BL + b
                f_last = hsf[l][:, cols].T.reshape(H)       # [jc,128] -> flat
                b_last = lastb[l][:, jcs * BL + b].T.reshape(H)
                out[g, (2 * l) * H:(2 * l + 1) * H] = f_last
                out[g, (2 * l + 1) * H:(2 * l + 2) * H] = b_last
    out *= LEAK  # state stored as p = h/LEAK
    return out
